# revision 38
# baseline (speedup 1.0000x reference)
"""AgentCollisionLoss Trainium2 kernel.

Full inputs -> full output. Shards the N (sample) dim across 8 NeuronCores
(2 samples per core), computes the pairwise agent-collision loss on device,
and gathers the per-core (NL, B) losses into the full (B, N) output.

Device layout (per core):
  partition p = n_local*T + t            (104 rows)

Stage A: world-frame agent positions pg [P,2,B] and yaw unit vectors
  (sincos) from x + per-agent consts.

Stage B (quadratic-min): the 5 disks of an agent are colinear and equally
  spaced, so for a pair (i,j) the squared disk-pair distance is a quadratic
  in the integer disk offsets (a,b) in [-2,2]:
    d2(a,b) = |w + a*su*u_i - b*sv*u_j|^2,  w = pos_i - pos_j
  For fixed a the minimizing integer b is clamp(round(vertex)), exactly
  (1-D convex quadratic on an integer grid):
    m(a) = alpha + beta*a + gamma*a^2 + Av*(R - T(a))^2
    T(a)  = (dwv + a*su*duv)/sv          R = clamp(round(T), -2, 2)
    alpha = W2 - dwv^2
    beta  = 2*su*(dwu - dwv*duv)
    gamma = su^2*(1 - duv^2)
  with duv = u_i.u_j, dwu = w.u_i, dwv = w.u_j, W2 = |w|^2, Av = sv^2.
  pdist = alpha + min_a(beta*a + gamma*a^2 + Av*(R-T)^2), clamped at 0.
  alpha (the only large-magnitude term) stays fp32 and is added after the
  min; the whole [P,5,PP] chain runs in fp16 (validated: rel err 4e-4).
  The sign of w is irrelevant: flipping w maps (a,b) -> (-a,-b).

Pairs: per scene, circulant (i, j=(i+k) mod s) for k=1..s//2 and all i;
  for even s the k=s/2 column covers each pair twice, compensated by 0.5
  weights in the `mind` scatter matrix.  PP = sum s*(s//2).

Stage C: sqrt, penalty = relu(1 - dist/pd) on the packed pair list.
Stage D: time-decay-weighted sum over t via two f16 PE matmuls, moving
  mask and /B, DMA out [NL, 32].
"""

import os
import sys

import numpy as np

for _p in ("/opt/trn_rl_repo", "/root/.axon_site/_ro/trn_rl_repo"):
    if os.path.isdir(_p) and _p not in sys.path:
        sys.path.insert(0, _p)

import bass_rust
import concourse.bass as bass
import concourse.mybir as mybir
import concourse.tile as tile
from concourse.bass_utils import run_bass_kernel_spmd
from concourse.vector_clock import ScopedClock


def _split_drain_and_barrier(self, tick_clock, wait_clock):
    """Kernel-tail drain, one semaphore per drain instruction.

    The walrus build in this container rejects instructions carrying more
    than one embedded sync wait ("Too many sync wait commands"). Tile's
    stock tail emits a single drain waiting on the full global clock, so
    split it: one drain per nonzero proc tick. add_sem_waits elides waits
    the engine has already observed, so each drain carries exactly one.
    """
    gc = list(tick_clock.global_clock)
    engs = [self.nc.sync, self.nc.vector, self.nc.scalar, self.nc.gpsimd,
            self.nc.tensor]
    nd = 0
    for idx, tick in enumerate(gc):
        if tick <= 0:
            continue
        v = [0] * len(gc)
        v[idx] = tick
        d = engs[nd % len(engs)].drain()
        nd += 1
        wait_clock.add_sem_waits(
            d.ins, ScopedClock({None: bass_rust.VectorClock(v)})
        )
    self.nc.all_engine_barrier()
    assert self.sems is not None
    popped = self.nc._tile_sem_poison_stack.pop()
    assert popped is self._sem_poison
    # DMA queues still need their ring reset, but per-sem clears are
    # redundant here: walrus's own kernel-tail sweep zeroes every sem in
    # [3, 255] right after this block, which covers the tile sems.
    sems = list(self.sems.allocated().values())
    if sems:
        nums = sorted(s.num if hasattr(s, "num") else s for s in sems)
        import concourse.bass as _bass_mod
        for rng in _bass_mod.compact_to_ranges(nums):
            self.nc.gpsimd.dma_reset(rng)
        self.nc._state.prepend_free_semaphores(nums)
    # No trailing all_engine_barrier: the walrus tail sweep that follows is
    # per-engine in-order, and both dma_reset and its sem zeroing are
    # idempotent resets — the post-drain barrier above already fenced all
    # compute.


tile.TileContext._drain_and_barrier = _split_drain_and_barrier

# (probed: walrus --max-sem-num does not shrink the kernel-tail sem sweep
# and perturbs DMA queue startup; left at defaults)

B, N, T, D = 32, 16, 52, 5
NCORES = 8
NL = N // NCORES          # samples per core
P = NL * T                # partition rows per core
PH = 64                   # row-split DMA boundary (32-aligned partition)
BUFFER_DIST = 0.2
DECAY_RATE = 0.9
SPEED_TH = 0.5

F32 = mybir.dt.float32
F16 = mybir.dt.float16
PI = float(np.pi)
MAGIC = 12582912.0        # 1.5*2^23: f32 round-to-nearest-even via add/sub
MAGIC16 = 1536.0          # 1.5*2^10: f16 round via add/sub (|x| <= 512)


def _rects(scenes):
    """Uniform circulant rects per scene: (offset, size, K=s//2)."""
    return [(o, s, s // 2) for (o, s) in scenes if s // 2 >= 1]


# xinA column layout (stage-A-critical): x(3B) | geo(8B) | wmt(NL)
XO_GEO = 3 * B
XO_WMTA = XO_GEO + 8 * B
XWA = XO_WMTA + NL


def _xin_width_b(PP):
    # f16 const rows: TWOSU | SU2 | INVSV | SUDSV | AV | PRC
    return 6 * PP


def _build_nc(scenes, PP):
    """Build the SPMD Bass program. `scenes` = [(offset, size)]."""
    nc = bass.Bass()

    XWB = _xin_width_b(PP)
    xina = nc.dram_tensor("xina", [P, XWA], F32, kind="ExternalInput")
    xinb = nc.dram_tensor("xinb", [P, XWB], F16, kind="ExternalInput")
    mind = nc.dram_tensor("mind", [PP, B], F16, kind="ExternalInput")
    out = nc.dram_tensor("loss", [NL, B], F32, kind="ExternalOutput")

    rects = _rects(scenes)
    poffs = []
    po = 0
    for (o, s, K) in rects:
        poffs.append(po)
        po += s * K
    assert po == PP

    XO_TWOSU = 0
    XO_SU2 = PP
    XO_INVSV = 2 * PP
    XO_SUDSV = 3 * PP
    XO_AV = 4 * PP
    XO_PRC = 5 * PP

    with tile.TileContext(nc) as tc:
        with (
            tc.tile_pool(name="singles", bufs=1) as singles,
            tc.tile_pool(name="small", bufs=1) as small,
            tc.tile_pool(name="psum", bufs=1, space="PSUM") as psum,
        ):
            # ---- loads: xina split by row-halves on two queues so the
            # stage-A-critical data lands sooner ----
            xta = singles.tile([P, XWA], F32)
            nc.sync.dma_start(out=xta[0:PH, :], in_=xina[0:PH, :])
            nc.scalar.dma_start(out=xta[PH:P, :], in_=xina[PH:P, :])
            xtb = singles.tile([P, XWB], F16)
            nc.gpsimd.dma_start(out=xtb[:], in_=xinb[:])
            mindt = singles.tile([PP, B], F16)
            nc.sync.dma_start(out=mindt[:], in_=mind[:])

            # RAMP[p, a, q] = a-2 (f16), materialized early on idle gpsimd
            # (before any gpsimd op that waits on a DMA, so the memsets run
            # immediately after the preamble).
            ramp = singles.tile([P, 5, PP], F16, tag="ramp")
            for a in range(5):
                nc.gpsimd.memset(ramp[:, a, :], float(a - 2))

            # Pre-touch DMA'd tiles per consuming engine (one DMA-queue sem
            # wait per touch; this walrus rejects >1 embedded sync wait).
            tch0 = singles.tile([P, 1], F32, tag="tch0")
            nc.vector.tensor_copy(out=tch0[0:PH, :], in_=xta[0:PH, 0:1])
            nc.vector.tensor_copy(out=tch0[PH:P, :], in_=xta[PH:P, 0:1])
            tchr = singles.tile([P, 1], F16, tag="tchr")
            nc.vector.tensor_copy(out=tchr[:], in_=ramp[:, 4, 0:1])
            tchg = singles.tile([P, 1], F16, tag="tchg")
            nc.gpsimd.tensor_copy(out=tchg[:], in_=xtb[:, 0:1])

            gA = xta[:, XO_GEO + 0 * B : XO_GEO + 2 * B]
            gT = xta[:, XO_GEO + 4 * B : XO_GEO + 6 * B]
            shifts2 = xta[:, XO_GEO + 6 * B : XO_GEO + 8 * B]
            x0 = xta[:, 0:B]
            yw = xta[:, 2 * B : 3 * B]

            def rep2(apx, w):
                return bass.AP(tensor=apx.tensor, offset=apx.offset,
                               ap=[apx.ap[0], [0, 2], [1, w]])

            # ---- stage A ----
            u2 = small.tile([P, 2, B], F32)
            nc.vector.scalar_tensor_tensor(
                out=u2[:], in0=rep2(yw, B), scalar=1.0 / (2.0 * PI),
                in1=shifts2.rearrange("p (c i) -> p c i", c=2),
                op0=mybir.AluOpType.mult, op1=mybir.AluOpType.add)
            kf = small.tile([P, 2, B], F32)
            nc.vector.tensor_scalar(
                out=kf[:], in0=u2[:], scalar1=MAGIC, scalar2=MAGIC,
                op0=mybir.AluOpType.add, op1=mybir.AluOpType.subtract)
            fr = small.tile([P, 2, B], F32)
            nc.vector.tensor_sub(fr[:], u2[:], kf[:])

            # G[p, q, i]: q=0 px, 1 py, 2 ux(cos), 3 uy(sin).
            # Sin writes rows 2/3 directly (c=0 sin-arg -> row 3, c=1
            # cos-arg -> row 2); the pos_g adds accumulate into rows 0/1.
            g = singles.tile([P, 4, B], F32, tag="g")
            gf = g[:].rearrange("p q i -> p (q i)")
            gp = gf.ap[0]
            eg = gf.ap[-1][0]
            sin_out = bass.AP(tensor=gf.tensor,
                              offset=gf.offset + 3 * B * eg,
                              ap=[gp, [-B * eg, 2], [eg, B]])
            nc.scalar.activation(out=sin_out, in_=fr[:],
                                 func=mybir.ActivationFunctionType.Sin,
                                 bias=0.0, scale=2.0 * PI)

            # pos_g into g[:, 0:2, :]
            m12 = small.tile([P, 2, 2, B], F32)
            xx = bass.AP(tensor=xta.tensor, offset=x0.offset,
                         ap=[x0.ap[0], [B, 2], [0, 2], [1, B]])
            gAB = bass.AP(tensor=xta.tensor, offset=gA.offset,
                          ap=[gA.ap[0], [2 * B, 2], [B, 2], [1, B]])
            nc.vector.tensor_mul(m12[:], xx, gAB)
            pg = g[:, 0:2, :]
            nc.vector.tensor_add(pg, m12[:, 0], m12[:, 1])
            nc.vector.tensor_add(pg, pg,
                                 gT.rearrange("p (c i) -> p c i", c=2))
            # DVE touch of g's ACT-written rows so the GI gathers carry
            # only their own-engine self-wait
            tchd2 = singles.tile([P, 1], F32, tag="tchd2")
            nc.vector.tensor_copy(
                out=tchd2[:],
                in_=bass.AP(tensor=gf.tensor,
                            offset=gf.offset + 2 * B * eg,
                            ap=[gp, [eg, 1]]))

            # ACT touch of g's DVE-written rows so the G2 copies carry only
            # their own-engine (sin) self-wait
            tcha = singles.tile([P, 1], F32, tag="tcha")
            nc.scalar.activation(out=tcha[:], in_=g[:, 0, 0:1],
                                 func=mybir.ActivationFunctionType.Copy)

            # doubled per-scene agent lists (wrap j = (i+k) mod s) — on ACT
            DBL = 2 * B
            g2 = singles.tile([P, 4, DBL], F32, tag="g2")
            g2f = g2[:].rearrange("p q i -> p (q i)")
            g2p = g2f.ap[0]
            e2 = g2f.ap[-1][0]
            dbl_off = {}
            do_ = 0
            for (o, s) in scenes:
                dbl_off[o] = do_
                in_ap = bass.AP(tensor=gf.tensor,
                                offset=gf.offset + o * eg,
                                ap=[gp, [B * eg, 4], [0, 2], [eg, s]])
                out_ap = bass.AP(tensor=g2f.tensor,
                                 offset=g2f.offset + do_ * e2,
                                 ap=[g2p, [DBL * e2, 4], [s * e2, 2],
                                     [e2, s]])
                nc.scalar.activation(out=out_ap, in_=in_ap,
                                     func=mybir.ActivationFunctionType.Copy)
                do_ += 2 * s

            # ---- gather materialization: GI/GJ [P, 4, PP] (ACT) ----
            gi = singles.tile([P, 4, PP], F32, tag="gi")
            gj = singles.tile([P, 4, PP], F32, tag="gj")
            gif = gi[:].rearrange("p q i -> p (q i)")
            gjf = gj[:].rearrange("p q i -> p (q i)")
            egi = gif.ap[-1][0]
            egj = gjf.ap[-1][0]
            for idx, (o, s, K) in enumerate(rects):
                poff = poffs[idx]
                out_i = bass.AP(tensor=gif.tensor,
                                offset=gif.offset + poff * egi,
                                ap=[gif.ap[0], [PP * egi, 4], [K * egi, s],
                                    [egi, K]])
                in_i = bass.AP(tensor=gf.tensor,
                               offset=gf.offset + o * eg,
                               ap=[gp, [B * eg, 4], [eg, s], [0, K]])
                nc.vector.tensor_copy(out=out_i, in_=in_i)
            for idx, (o, s, K) in enumerate(rects):
                poff = poffs[idx]
                out_j = bass.AP(tensor=gjf.tensor,
                                offset=gjf.offset + poff * egj,
                                ap=[gjf.ap[0], [PP * egj, 4], [K * egj, s],
                                    [egj, K]])
                in_j = bass.AP(
                    tensor=g2f.tensor,
                    offset=g2f.offset + (dbl_off[o] + 1) * e2,
                    ap=[g2p, [DBL * e2, 4], [e2, s], [e2, K]])
                nc.vector.tensor_copy(out=out_j, in_=in_j)

            # ---- stage B packed ops over [P, 2, PP] ----
            wxy = singles.tile([P, 2, PP], F32, tag="wxy")
            nc.vector.tensor_sub(wxy[:], gi[:, 0:2, :], gj[:, 0:2, :])
            # mvv[q=0] = u_i*u_j products, mvv[q=1] = w*u_j products, so one
            # add produces dw (row0=duv, row1=dwv) in a single op
            mvv = singles.tile([P, 2, 2, PP], F32, tag="mvv")
            nc.vector.tensor_mul(mvv[:, 0], gi[:, 2:4, :], gj[:, 2:4, :])
            nc.vector.tensor_mul(mvv[:, 1], wxy[:], gj[:, 2:4, :])
            mwu = singles.tile([P, 2, PP], F32, tag="mwu")
            nc.vector.tensor_mul(mwu[:], wxy[:], gi[:, 2:4, :])

            dw = singles.tile([P, 2, PP], F32, tag="dw")
            nc.vector.tensor_add(dw[:], mvv[:, :, 0, :], mvv[:, :, 1, :])
            du = singles.tile([P, PP], F32, tag="du")
            nc.vector.tensor_add(du[:], mwu[:, 0, :], mwu[:, 1, :])

            # squares on ACT
            sqw = singles.tile([P, 2, PP], F32, tag="sqw")
            nc.scalar.activation(out=sqw[:], in_=wxy[:],
                                 func=mybir.ActivationFunctionType.Square)
            dw2 = singles.tile([P, 2, PP], F32, tag="dw2")
            nc.scalar.activation(out=dw2[:], in_=dw[:],
                                 func=mybir.ActivationFunctionType.Square)

            # off-critical per-pair scalars on gpsimd (consumed late):
            # alpha (f32, post-min) and gamma (f16, poly)
            tchs = singles.tile([P, 1], F32, tag="tchs")
            nc.gpsimd.tensor_copy(out=tchs[:], in_=sqw[:, 0, 0:1])
            w2t = singles.tile([P, PP], F32, tag="w2t")
            nc.gpsimd.tensor_add(w2t[:], sqw[:, 0, :], sqw[:, 1, :])
            # e1t (no own-engine input) carries the dw2 ACT wait; alph then
            # needs only its own-engine self-wait.
            e1t = singles.tile([P, PP], F32, tag="e1t")
            nc.gpsimd.tensor_scalar(
                out=e1t[:], in0=dw2[:, 0, :], scalar1=-1.0, scalar2=1.0,
                op0=mybir.AluOpType.mult, op1=mybir.AluOpType.add)
            alph = singles.tile([P, PP], F32, tag="alph")
            nc.gpsimd.tensor_sub(alph[:], w2t[:], dw2[:, 1, :])
            gam = singles.tile([P, PP], F16, tag="gam")
            nc.gpsimd.tensor_mul(gam[:], e1t[:],
                                 xtb[:, XO_SU2 : XO_SU2 + PP])

            # chain feeders on DVE (f16 outputs); avh doubles as the DVE
            # toucher for the xtb DMA (no own-engine input -> one wait)
            avh = singles.tile([P, PP], F16, tag="avh")
            nc.vector.tensor_copy(out=avh[:],
                                  in_=xtb[:, XO_AV : XO_AV + PP])
            p1 = singles.tile([P, PP], F32, tag="p1")
            nc.vector.tensor_mul(p1[:], dw[:, 0, :], dw[:, 1, :])
            zt = singles.tile([P, PP], F32, tag="zt")
            nc.vector.scalar_tensor_tensor(
                out=zt[:], in0=p1[:], scalar=-1.0, in1=du[:],
                op0=mybir.AluOpType.mult, op1=mybir.AluOpType.add)
            bet = singles.tile([P, PP], F16, tag="bet")
            nc.vector.tensor_mul(bet[:], zt[:],
                                 xtb[:, XO_TWOSU : XO_TWOSU + PP])
            tb = singles.tile([P, PP], F16, tag="tb")
            nc.vector.tensor_mul(tb[:], dw[:, 1, :],
                                 xtb[:, XO_INVSV : XO_INVSV + PP])
            tsl = singles.tile([P, PP], F16, tag="tsl")
            nc.vector.tensor_mul(tsl[:], dw[:, 0, :],
                                 xtb[:, XO_SUDSV : XO_SUDSV + PP])

            def brow(apx):
                """[P, 5, PP] view reading a [P, PP] row 5x (a-broadcast)"""
                return bass.AP(tensor=apx.tensor, offset=apx.offset,
                               ap=[apx.ap[0], [0, 5], [apx.ap[-1][0], PP]])

            # ---- Q-stage over [P, 5, PP], all f16 ----
            tt = singles.tile([P, 5, PP], F16, tag="tt")
            nc.vector.tensor_mul(tt[:], brow(tsl[:]), ramp[:])
            nc.vector.tensor_add(tt[:], tt[:], brow(tb[:]))
            # round then clamp (equivalent: grid ends are integers), DVE ts
            rt = singles.tile([P, 5, PP], F16, tag="rt")
            nc.vector.tensor_scalar(
                out=rt[:], in0=tt[:], scalar1=MAGIC, scalar2=MAGIC,
                op0=mybir.AluOpType.add, op1=mybir.AluOpType.subtract)
            ct = singles.tile([P, 5, PP], F16, tag="ct")
            nc.vector.tensor_scalar(
                out=ct[:], in0=rt[:], scalar1=2.0, scalar2=-2.0,
                op0=mybir.AluOpType.min, op1=mybir.AluOpType.max)
            ddt = singles.tile([P, 5, PP], F16, tag="ddt")
            nc.vector.tensor_sub(ddt[:], ct[:], tt[:])
            dd2 = singles.tile([P, 5, PP], F16, tag="dd2")
            nc.vector.tensor_mul(dd2[:], ddt[:], ddt[:])
            vv = singles.tile([P, 5, PP], F16, tag="vv")
            nc.vector.tensor_mul(vv[:], dd2[:], brow(avh[:]))
            # poly+merge: M = (gamma*RA + beta)*RA + V   (alpha added later)
            h1 = singles.tile([P, 5, PP], F16, tag="h1")
            nc.vector.tensor_mul(h1[:], brow(gam[:]), ramp[:])
            nc.vector.tensor_add(h1[:], h1[:], brow(bet[:]))
            nc.vector.tensor_mul(h1[:], h1[:], ramp[:])
            mm = singles.tile([P, 5, PP], F16, tag="mm")
            nc.vector.tensor_add(mm[:], h1[:], vv[:])

            # min over a (tree), + alpha (f32), clamp at 0
            t01 = singles.tile([P, 2, PP], F16, tag="t01")
            nc.vector.tensor_tensor(out=t01[:], in0=mm[:, 0:2, :],
                                    in1=mm[:, 2:4, :],
                                    op=mybir.AluOpType.min)
            t2 = singles.tile([P, PP], F16, tag="t2")
            nc.vector.tensor_tensor(out=t2[:], in0=t01[:, 0, :],
                                    in1=t01[:, 1, :],
                                    op=mybir.AluOpType.min)
            m5 = singles.tile([P, PP], F16, tag="m5")
            nc.vector.tensor_tensor(out=m5[:], in0=t2[:], in1=mm[:, 4, :],
                                    op=mybir.AluOpType.min)
            pda = singles.tile([P, PP], F32, tag="pda")
            nc.vector.tensor_add(pda[:], m5[:], alph[:])
            pdist = singles.tile([P, PP], F32, tag="pdist")
            nc.vector.tensor_scalar(
                out=pdist[:], in0=pda[:], scalar1=0.0, scalar2=0.0,
                op0=mybir.AluOpType.max, op1=mybir.AluOpType.add)

            # ---- fused stage C ----
            dist = small.tile([P, PP], F32, tag="dist")
            nc.scalar.activation(out=dist[:], in_=pdist[:],
                                 func=mybir.ActivationFunctionType.Sqrt)
            rr = small.tile([P, PP], F32, tag="rr")
            nc.vector.scalar_tensor_tensor(
                out=rr[:], in0=dist[:], scalar=-1.0,
                in1=xtb[:, XO_PRC : XO_PRC + PP],
                op0=mybir.AluOpType.mult, op1=mybir.AluOpType.mult)
            pen = small.tile([P, PP], F16, tag="pen")
            nc.vector.tensor_scalar(
                out=pen[:], in0=rr[:], scalar1=1.0, scalar2=0.0,
                op0=mybir.AluOpType.add, op1=mybir.AluOpType.max)

            # ---- stage D ----
            w2f = singles.tile([P, NL], F16, tag="w2f")
            nc.vector.tensor_copy(out=w2f[:],
                                  in_=xta[:, XO_WMTA : XO_WMTA + NL])
            mindc = small.tile([PP, B], F16, tag="mindc")
            nc.gpsimd.tensor_copy(out=mindc[:], in_=mindt[:])
            ps1 = psum.tile([PP, 512], F32)
            nc.tensor.matmul(ps1[:, 0:NL], pen[:], w2f[:],
                             start=True, stop=True)
            pairsum = small.tile([PP, NL], F16, tag="pairsum")
            nc.vector.tensor_copy(out=pairsum[:], in_=ps1[:, 0:NL])
            ps2 = psum.tile([NL, 512], F32)
            nc.tensor.matmul(ps2[:, 0:B], pairsum[:], mindc[:],
                             start=True, stop=True)
            lout = small.tile([NL, B], F32, tag="lout")
            nc.vector.tensor_copy(out=lout[:], in_=ps2[:, 0:B])
            nc.sync.dma_start(out=out[:], in_=lout[:])

    return nc


def _prepare(inputs):
    x = np.ascontiguousarray(inputs["x"], dtype=np.float32)
    extent = np.asarray(inputs["extent"], dtype=np.float32)
    wfa = np.asarray(inputs["world_from_agent"], dtype=np.float32)
    speed = np.asarray(inputs["curr_speed"], dtype=np.float32)
    scene = np.asarray(inputs["scene_index"])

    R = wfa[:, :2, :2]
    tr = wfa[:, :2, 2]
    yaw_off = np.arctan2(R[:, 1, 0], R[:, 0, 0]).astype(np.float32)
    agt_rad = extent[:, 1] / 2.0
    cmax = extent[:, 0] / 2.0 - agt_rad
    su = (cmax / 2.0).astype(np.float32)          # disk spacing
    pd = (agt_rad[:, None] + agt_rad[None, :] + BUFFER_DIST).astype(np.float32)
    moving = (np.abs(speed) > SPEED_TH)

    _, starts, counts = np.unique(scene, return_index=True, return_counts=True)
    scenes = [(int(o), int(s)) for o, s in zip(starts, counts)]
    assert sum(s for _, s in scenes) == B
    for o, s in scenes:
        assert (scene[o : o + s] == scene[o]).all()

    pairs_i, pairs_j, pairs_w = [], [], []
    for (o, s, K) in _rects(scenes):
        for i in range(s):
            for k in range(1, K + 1):
                pairs_i.append(o + i)
                pairs_j.append(o + (i + k) % s)
                pairs_w.append(0.5 if (s % 2 == 0 and k == s // 2) else 1.0)
    pairs_i = np.array(pairs_i)
    pairs_j = np.array(pairs_j)
    pairs_w = np.array(pairs_w, dtype=np.float32)
    PP = len(pairs_i)

    sui = su[pairs_i]
    svj = su[pairs_j]
    const_rows = np.concatenate([
        2.0 * sui,                      # TWOSU
        sui * sui,                      # SU2
        1.0 / svj,                      # INVSV
        sui / svj,                      # SUDSV
        svj * svj,                      # AV
        1.0 / pd[pairs_i, pairs_j],     # PRC
    ]).astype(np.float32)

    mind_arr = np.zeros((PP, B), dtype=np.float16)
    mv = moving.astype(np.float32)
    for q in range(PP):
        mind_arr[q, pairs_i[q]] = mv[pairs_i[q]] * pairs_w[q]
        mind_arr[q, pairs_j[q]] = mv[pairs_j[q]] * pairs_w[q]

    twopi = 2.0 * np.pi
    geo = np.concatenate([
        R[:, 0, 0], R[:, 1, 0],          # gA
        R[:, 0, 1], R[:, 1, 1],          # gB
        tr[:, 0], tr[:, 1],              # gT
        2.0 + yaw_off / twopi, 2.25 + yaw_off / twopi,  # shifts2
    ]).astype(np.float32)

    w = DECAY_RATE ** np.arange(T, dtype=np.float32)
    w = w / w.sum()
    wmt = np.zeros((P, NL), dtype=np.float32)
    for nl in range(NL):
        wmt[nl * T : (nl + 1) * T, nl] = w / B

    XWB = _xin_width_b(PP)
    xinb_row = np.empty((P, XWB), dtype=np.float16)
    xinb_row[:, :] = const_rows[None, :].astype(np.float16)
    in_maps = []
    for c in range(NCORES):
        xs = x[:, c * NL : (c + 1) * NL, :, :]          # (B, NL, T, 6)
        xs = xs[..., [0, 1, 3]]                          # (B, NL, T, 3)
        xdat = xs.transpose(1, 2, 3, 0).reshape(P, 3 * B)
        xina = np.empty((P, XWA), dtype=np.float32)
        xina[:, 0 : 3 * B] = xdat
        xina[:, XO_GEO : XO_WMTA] = geo[None, :]
        xina[:, XO_WMTA:] = wmt
        in_maps.append({"xina": xina, "xinb": xinb_row, "mind": mind_arr})

    return scenes, PP, in_maps, moving


_CACHE = {}


def _get_nc(scenes, PP):
    key = (tuple(scenes), PP)
    if key not in _CACHE:
        _CACHE[key] = _build_nc(scenes, PP)
    return _CACHE[key]


def _run(inputs, trace=False):
    scenes, PP, in_maps, moving = _prepare(inputs)
    nc = _get_nc(scenes, PP)
    res = run_bass_kernel_spmd(nc, in_maps, core_ids=list(range(NCORES)),
                               trace=trace)
    outf = np.zeros((B, N), dtype=np.float32)
    for c in range(NCORES):
        lc = res.results[c]["loss"]                      # (NL, B)
        for nl in range(NL):
            outf[:, c * NL + nl] = lc[nl]
    return outf, res


def kernel(**inputs):
    outf, _ = _run(inputs, trace=False)
    return outf


def _ensure_ntff_hook():
    """Register the axon NTFF profile hook if the container's antenv lacks it."""
    try:
        from antenv.axon_hooks import get_axon_ntff_profile_hook  # noqa: F401
        return
    except ImportError:
        pass
    import types

    if "/root/.axon_site" not in sys.path:
        sys.path.insert(0, "/root/.axon_site")
    from trn_agent_boot.trn_boot import _ntff_profile_via_ctypes

    hook = _ntff_profile_via_ctypes("/opt/axon/libaxon_pjrt.so")
    mod = types.ModuleType("antenv.axon_hooks")
    mod.get_axon_ntff_profile_hook = lambda: hook
    mod.set_axon_ntff_profile_hook = lambda h: None
    sys.modules["antenv.axon_hooks"] = mod


def run_traced(inputs):
    """Correctness output + profiled exec time (ns) via NTFF trace."""
    _ensure_ntff_hook()
    outf, res = _run(inputs, trace=True)
    return outf, res.exec_time_ns


# revision 39
# speedup vs baseline: 1.0007x; 1.0007x over previous
"""AgentCollisionLoss Trainium2 kernel.

Full inputs -> full output. Shards the N (sample) dim across 8 NeuronCores
(2 samples per core), computes the pairwise agent-collision loss on device,
and gathers the per-core (NL, B) losses into the full (B, N) output.

Device layout (per core):
  partition p = n_local*T + t            (104 rows)

Stage A: world-frame agent positions pg [P,2,B] and yaw unit vectors
  (sincos) from x + per-agent consts.

Stage B (quadratic-min): the 5 disks of an agent are colinear and equally
  spaced, so for a pair (i,j) the squared disk-pair distance is a quadratic
  in the integer disk offsets (a,b) in [-2,2]:
    d2(a,b) = |w + a*su*u_i - b*sv*u_j|^2,  w = pos_i - pos_j
  For fixed a the minimizing integer b is clamp(round(vertex)), exactly
  (1-D convex quadratic on an integer grid):
    m(a) = alpha + beta*a + gamma*a^2 + Av*(R - T(a))^2
    T(a)  = (dwv + a*su*duv)/sv          R = clamp(round(T), -2, 2)
    alpha = W2 - dwv^2
    beta  = 2*su*(dwu - dwv*duv)
    gamma = su^2*(1 - duv^2)
  with duv = u_i.u_j, dwu = w.u_i, dwv = w.u_j, W2 = |w|^2, Av = sv^2.
  pdist = alpha + min_a(beta*a + gamma*a^2 + Av*(R-T)^2), clamped at 0.
  alpha (the only large-magnitude term) stays fp32 and is added after the
  min; the whole [P,5,PP] chain runs in fp16 (validated: rel err 4e-4).
  The sign of w is irrelevant: flipping w maps (a,b) -> (-a,-b).

Pairs: per scene, circulant (i, j=(i+k) mod s) for k=1..s//2 and all i;
  for even s the k=s/2 column covers each pair twice, compensated by 0.5
  weights in the `mind` scatter matrix.  PP = sum s*(s//2).

Stage C: sqrt, penalty = relu(1 - dist/pd) on the packed pair list.
Stage D: time-decay-weighted sum over t via two f16 PE matmuls, moving
  mask and /B, DMA out [NL, 32].
"""

import os
import sys

import numpy as np

for _p in ("/opt/trn_rl_repo", "/root/.axon_site/_ro/trn_rl_repo"):
    if os.path.isdir(_p) and _p not in sys.path:
        sys.path.insert(0, _p)

import bass_rust
import concourse.bass as bass
import concourse.mybir as mybir
import concourse.tile as tile
from concourse.bass_utils import run_bass_kernel_spmd
from concourse.vector_clock import ScopedClock


def _split_drain_and_barrier(self, tick_clock, wait_clock):
    """Kernel-tail drain, one semaphore per drain instruction.

    The walrus build in this container rejects instructions carrying more
    than one embedded sync wait ("Too many sync wait commands"). Tile's
    stock tail emits a single drain waiting on the full global clock, so
    split it: one drain per nonzero proc tick. add_sem_waits elides waits
    the engine has already observed, so each drain carries exactly one.
    """
    gc = list(tick_clock.global_clock)
    engs = [self.nc.sync, self.nc.vector, self.nc.scalar, self.nc.gpsimd,
            self.nc.tensor]
    nd = 0
    for idx, tick in enumerate(gc):
        if tick <= 0:
            continue
        v = [0] * len(gc)
        v[idx] = tick
        d = engs[nd % len(engs)].drain()
        nd += 1
        wait_clock.add_sem_waits(
            d.ins, ScopedClock({None: bass_rust.VectorClock(v)})
        )
    self.nc.all_engine_barrier()
    assert self.sems is not None
    popped = self.nc._tile_sem_poison_stack.pop()
    assert popped is self._sem_poison
    # DMA queues still need their ring reset, but per-sem clears are
    # redundant here: walrus's own kernel-tail sweep zeroes every sem in
    # [3, 255] right after this block, which covers the tile sems.
    sems = list(self.sems.allocated().values())
    if sems:
        nums = sorted(s.num if hasattr(s, "num") else s for s in sems)
        import concourse.bass as _bass_mod
        for rng in _bass_mod.compact_to_ranges(nums):
            self.nc.gpsimd.dma_reset(rng)
        self.nc._state.prepend_free_semaphores(nums)
    # No trailing all_engine_barrier: the walrus tail sweep that follows is
    # per-engine in-order, and both dma_reset and its sem zeroing are
    # idempotent resets — the post-drain barrier above already fenced all
    # compute.


tile.TileContext._drain_and_barrier = _split_drain_and_barrier

# (probed: walrus --max-sem-num does not shrink the kernel-tail sem sweep
# and perturbs DMA queue startup; left at defaults)

B, N, T, D = 32, 16, 52, 5
NCORES = 8
NL = N // NCORES          # samples per core
P = NL * T                # partition rows per core
PH = 64                   # row-split DMA boundary (32-aligned partition)
BUFFER_DIST = 0.2
DECAY_RATE = 0.9
SPEED_TH = 0.5

F32 = mybir.dt.float32
F16 = mybir.dt.float16
PI = float(np.pi)
MAGIC = 12582912.0        # 1.5*2^23: f32 round-to-nearest-even via add/sub
MAGIC16 = 1536.0          # 1.5*2^10: f16 round via add/sub (|x| <= 512)


def _rects(scenes):
    """Uniform circulant rects per scene: (offset, size, K=s//2)."""
    return [(o, s, s // 2) for (o, s) in scenes if s // 2 >= 1]


# xinA column layout (stage-A-critical): x(3B) | geo(8B) | wmt(NL)
XO_GEO = 3 * B
XO_WMTA = XO_GEO + 8 * B
XWA = XO_WMTA + NL


def _xin_width_b(PP):
    # f16 const rows: TWOSU | SU2 | INVSV | SUDSV | AV | PRC
    return 6 * PP


def _build_nc(scenes, PP):
    """Build the SPMD Bass program. `scenes` = [(offset, size)]."""
    nc = bass.Bass()

    XWB = _xin_width_b(PP)
    xina = nc.dram_tensor("xina", [P, XWA], F32, kind="ExternalInput")
    xinb = nc.dram_tensor("xinb", [P, XWB], F16, kind="ExternalInput")
    mind = nc.dram_tensor("mind", [PP, B], F16, kind="ExternalInput")
    out = nc.dram_tensor("loss", [NL, B], F32, kind="ExternalOutput")

    rects = _rects(scenes)
    poffs = []
    po = 0
    for (o, s, K) in rects:
        poffs.append(po)
        po += s * K
    assert po == PP

    XO_TWOSU = 0
    XO_SU2 = PP
    XO_INVSV = 2 * PP
    XO_SUDSV = 3 * PP
    XO_AV = 4 * PP
    XO_PRC = 5 * PP

    with tile.TileContext(nc) as tc:
        with (
            tc.tile_pool(name="singles", bufs=1) as singles,
            tc.tile_pool(name="small", bufs=1) as small,
            tc.tile_pool(name="psum", bufs=1, space="PSUM") as psum,
        ):
            # ---- loads: xina split by row-halves on two queues so the
            # stage-A-critical data lands sooner ----
            xta = singles.tile([P, XWA], F32)
            nc.sync.dma_start(out=xta[0:PH, :], in_=xina[0:PH, :])
            nc.scalar.dma_start(out=xta[PH:P, :], in_=xina[PH:P, :])
            xtb = singles.tile([P, XWB], F16)
            nc.gpsimd.dma_start(out=xtb[:], in_=xinb[:])
            mindt = singles.tile([PP, B], F16)
            nc.sync.dma_start(out=mindt[:], in_=mind[:])

            # RAMP[p, a, q] = a-2 (f16) on DVE: it is idle during the DMA
            # wait, and DVE-own ramp removes a cross-engine wait from the
            # Q-chain.
            ramp = singles.tile([P, 5, PP], F16, tag="ramp")
            for a in range(5):
                nc.vector.memset(ramp[:, a, :], float(a - 2))

            # Pre-touch DMA'd tiles per consuming engine (one DMA-queue sem
            # wait per touch; this walrus rejects >1 embedded sync wait).
            tch0 = singles.tile([P, 1], F32, tag="tch0")
            nc.vector.tensor_copy(out=tch0[0:PH, :], in_=xta[0:PH, 0:1])
            nc.vector.tensor_copy(out=tch0[PH:P, :], in_=xta[PH:P, 0:1])
            tchg = singles.tile([P, 1], F16, tag="tchg")
            nc.gpsimd.tensor_copy(out=tchg[:], in_=xtb[:, 0:1])

            gA = xta[:, XO_GEO + 0 * B : XO_GEO + 2 * B]
            gT = xta[:, XO_GEO + 4 * B : XO_GEO + 6 * B]
            shifts2 = xta[:, XO_GEO + 6 * B : XO_GEO + 8 * B]
            x0 = xta[:, 0:B]
            yw = xta[:, 2 * B : 3 * B]

            def rep2(apx, w):
                return bass.AP(tensor=apx.tensor, offset=apx.offset,
                               ap=[apx.ap[0], [0, 2], [1, w]])

            # ---- stage A ----
            u2 = small.tile([P, 2, B], F32)
            nc.vector.scalar_tensor_tensor(
                out=u2[:], in0=rep2(yw, B), scalar=1.0 / (2.0 * PI),
                in1=shifts2.rearrange("p (c i) -> p c i", c=2),
                op0=mybir.AluOpType.mult, op1=mybir.AluOpType.add)
            kf = small.tile([P, 2, B], F32)
            nc.vector.tensor_scalar(
                out=kf[:], in0=u2[:], scalar1=MAGIC, scalar2=MAGIC,
                op0=mybir.AluOpType.add, op1=mybir.AluOpType.subtract)
            fr = small.tile([P, 2, B], F32)
            nc.vector.tensor_sub(fr[:], u2[:], kf[:])

            # G[p, q, i]: q=0 px, 1 py, 2 ux(cos), 3 uy(sin).
            # Sin writes rows 2/3 directly (c=0 sin-arg -> row 3, c=1
            # cos-arg -> row 2); the pos_g adds accumulate into rows 0/1.
            g = singles.tile([P, 4, B], F32, tag="g")
            gf = g[:].rearrange("p q i -> p (q i)")
            gp = gf.ap[0]
            eg = gf.ap[-1][0]
            sin_out = bass.AP(tensor=gf.tensor,
                              offset=gf.offset + 3 * B * eg,
                              ap=[gp, [-B * eg, 2], [eg, B]])
            nc.scalar.activation(out=sin_out, in_=fr[:],
                                 func=mybir.ActivationFunctionType.Sin,
                                 bias=0.0, scale=2.0 * PI)

            # pos_g into g[:, 0:2, :]
            m12 = small.tile([P, 2, 2, B], F32)
            xx = bass.AP(tensor=xta.tensor, offset=x0.offset,
                         ap=[x0.ap[0], [B, 2], [0, 2], [1, B]])
            gAB = bass.AP(tensor=xta.tensor, offset=gA.offset,
                          ap=[gA.ap[0], [2 * B, 2], [B, 2], [1, B]])
            nc.vector.tensor_mul(m12[:], xx, gAB)
            pg = g[:, 0:2, :]
            nc.vector.tensor_add(pg, m12[:, 0], m12[:, 1])
            nc.vector.tensor_add(pg, pg,
                                 gT.rearrange("p (c i) -> p c i", c=2))
            # DVE touch of g's ACT-written rows so the GI gathers carry
            # only their own-engine self-wait
            tchd2 = singles.tile([P, 1], F32, tag="tchd2")
            nc.vector.tensor_copy(
                out=tchd2[:],
                in_=bass.AP(tensor=gf.tensor,
                            offset=gf.offset + 2 * B * eg,
                            ap=[gp, [eg, 1]]))

            # ACT touch of g's DVE-written rows so the G2 copies carry only
            # their own-engine (sin) self-wait
            tcha = singles.tile([P, 1], F32, tag="tcha")
            nc.scalar.activation(out=tcha[:], in_=g[:, 0, 0:1],
                                 func=mybir.ActivationFunctionType.Copy)

            # doubled per-scene agent lists (wrap j = (i+k) mod s) — on ACT
            DBL = 2 * B
            g2 = singles.tile([P, 4, DBL], F32, tag="g2")
            g2f = g2[:].rearrange("p q i -> p (q i)")
            g2p = g2f.ap[0]
            e2 = g2f.ap[-1][0]
            dbl_off = {}
            do_ = 0
            for (o, s) in scenes:
                dbl_off[o] = do_
                in_ap = bass.AP(tensor=gf.tensor,
                                offset=gf.offset + o * eg,
                                ap=[gp, [B * eg, 4], [0, 2], [eg, s]])
                out_ap = bass.AP(tensor=g2f.tensor,
                                 offset=g2f.offset + do_ * e2,
                                 ap=[g2p, [DBL * e2, 4], [s * e2, 2],
                                     [e2, s]])
                nc.scalar.activation(out=out_ap, in_=in_ap,
                                     func=mybir.ActivationFunctionType.Copy)
                do_ += 2 * s

            # ---- gather materialization: GI/GJ [P, 4, PP] (ACT) ----
            gi = singles.tile([P, 4, PP], F32, tag="gi")
            gj = singles.tile([P, 4, PP], F32, tag="gj")
            gif = gi[:].rearrange("p q i -> p (q i)")
            gjf = gj[:].rearrange("p q i -> p (q i)")
            egi = gif.ap[-1][0]
            egj = gjf.ap[-1][0]
            for idx, (o, s, K) in enumerate(rects):
                poff = poffs[idx]
                out_i = bass.AP(tensor=gif.tensor,
                                offset=gif.offset + poff * egi,
                                ap=[gif.ap[0], [PP * egi, 4], [K * egi, s],
                                    [egi, K]])
                in_i = bass.AP(tensor=gf.tensor,
                               offset=gf.offset + o * eg,
                               ap=[gp, [B * eg, 4], [eg, s], [0, K]])
                nc.vector.tensor_copy(out=out_i, in_=in_i)
            for idx, (o, s, K) in enumerate(rects):
                poff = poffs[idx]
                out_j = bass.AP(tensor=gjf.tensor,
                                offset=gjf.offset + poff * egj,
                                ap=[gjf.ap[0], [PP * egj, 4], [K * egj, s],
                                    [egj, K]])
                in_j = bass.AP(
                    tensor=g2f.tensor,
                    offset=g2f.offset + (dbl_off[o] + 1) * e2,
                    ap=[g2p, [DBL * e2, 4], [e2, s], [e2, K]])
                nc.vector.tensor_copy(out=out_j, in_=in_j)

            # ---- stage B packed ops over [P, 2, PP] ----
            wxy = singles.tile([P, 2, PP], F32, tag="wxy")
            nc.vector.tensor_sub(wxy[:], gi[:, 0:2, :], gj[:, 0:2, :])
            # mvv[q=0] = u_i*u_j products, mvv[q=1] = w*u_j products, so one
            # add produces dw (row0=duv, row1=dwv) in a single op
            mvv = singles.tile([P, 2, 2, PP], F32, tag="mvv")
            nc.vector.tensor_mul(mvv[:, 0], gi[:, 2:4, :], gj[:, 2:4, :])
            nc.vector.tensor_mul(mvv[:, 1], wxy[:], gj[:, 2:4, :])
            mwu = singles.tile([P, 2, PP], F32, tag="mwu")
            nc.vector.tensor_mul(mwu[:], wxy[:], gi[:, 2:4, :])

            dw = singles.tile([P, 2, PP], F32, tag="dw")
            nc.vector.tensor_add(dw[:], mvv[:, :, 0, :], mvv[:, :, 1, :])
            du = singles.tile([P, PP], F32, tag="du")
            nc.vector.tensor_add(du[:], mwu[:, 0, :], mwu[:, 1, :])

            # squares on ACT
            sqw = singles.tile([P, 2, PP], F32, tag="sqw")
            nc.scalar.activation(out=sqw[:], in_=wxy[:],
                                 func=mybir.ActivationFunctionType.Square)
            dw2 = singles.tile([P, 2, PP], F32, tag="dw2")
            nc.scalar.activation(out=dw2[:], in_=dw[:],
                                 func=mybir.ActivationFunctionType.Square)

            # off-critical per-pair scalars on gpsimd (consumed late):
            # alpha (f32, post-min) and gamma (f16, poly)
            tchs = singles.tile([P, 1], F32, tag="tchs")
            nc.gpsimd.tensor_copy(out=tchs[:], in_=sqw[:, 0, 0:1])
            w2t = singles.tile([P, PP], F32, tag="w2t")
            nc.gpsimd.tensor_add(w2t[:], sqw[:, 0, :], sqw[:, 1, :])
            # e1t (no own-engine input) carries the dw2 ACT wait; alph then
            # needs only its own-engine self-wait.
            e1t = singles.tile([P, PP], F32, tag="e1t")
            nc.gpsimd.tensor_scalar(
                out=e1t[:], in0=dw2[:, 0, :], scalar1=-1.0, scalar2=1.0,
                op0=mybir.AluOpType.mult, op1=mybir.AluOpType.add)
            alph = singles.tile([P, PP], F32, tag="alph")
            nc.gpsimd.tensor_sub(alph[:], w2t[:], dw2[:, 1, :])
            gam = singles.tile([P, PP], F16, tag="gam")
            nc.gpsimd.tensor_mul(gam[:], e1t[:],
                                 xtb[:, XO_SU2 : XO_SU2 + PP])

            # chain feeders on DVE (f16 outputs); avh doubles as the DVE
            # toucher for the xtb DMA (no own-engine input -> one wait)
            avh = singles.tile([P, PP], F16, tag="avh")
            nc.vector.tensor_copy(out=avh[:],
                                  in_=xtb[:, XO_AV : XO_AV + PP])
            p1 = singles.tile([P, PP], F32, tag="p1")
            nc.vector.tensor_mul(p1[:], dw[:, 0, :], dw[:, 1, :])
            zt = singles.tile([P, PP], F32, tag="zt")
            nc.vector.scalar_tensor_tensor(
                out=zt[:], in0=p1[:], scalar=-1.0, in1=du[:],
                op0=mybir.AluOpType.mult, op1=mybir.AluOpType.add)
            bet = singles.tile([P, PP], F16, tag="bet")
            nc.vector.tensor_mul(bet[:], zt[:],
                                 xtb[:, XO_TWOSU : XO_TWOSU + PP])
            tb = singles.tile([P, PP], F16, tag="tb")
            nc.vector.tensor_mul(tb[:], dw[:, 1, :],
                                 xtb[:, XO_INVSV : XO_INVSV + PP])
            tsl = singles.tile([P, PP], F16, tag="tsl")
            nc.vector.tensor_mul(tsl[:], dw[:, 0, :],
                                 xtb[:, XO_SUDSV : XO_SUDSV + PP])

            def brow(apx):
                """[P, 5, PP] view reading a [P, PP] row 5x (a-broadcast)"""
                return bass.AP(tensor=apx.tensor, offset=apx.offset,
                               ap=[apx.ap[0], [0, 5], [apx.ap[-1][0], PP]])

            # ---- Q-stage over [P, 5, PP], all f16 ----
            tt = singles.tile([P, 5, PP], F16, tag="tt")
            nc.vector.tensor_mul(tt[:], brow(tsl[:]), ramp[:])
            nc.vector.tensor_add(tt[:], tt[:], brow(tb[:]))
            # round then clamp (equivalent: grid ends are integers), DVE ts
            rt = singles.tile([P, 5, PP], F16, tag="rt")
            nc.vector.tensor_scalar(
                out=rt[:], in0=tt[:], scalar1=MAGIC, scalar2=MAGIC,
                op0=mybir.AluOpType.add, op1=mybir.AluOpType.subtract)
            ct = singles.tile([P, 5, PP], F16, tag="ct")
            nc.vector.tensor_scalar(
                out=ct[:], in0=rt[:], scalar1=2.0, scalar2=-2.0,
                op0=mybir.AluOpType.min, op1=mybir.AluOpType.max)
            ddt = singles.tile([P, 5, PP], F16, tag="ddt")
            nc.vector.tensor_sub(ddt[:], ct[:], tt[:])
            dd2 = singles.tile([P, 5, PP], F16, tag="dd2")
            nc.vector.tensor_mul(dd2[:], ddt[:], ddt[:])
            vv = singles.tile([P, 5, PP], F16, tag="vv")
            nc.vector.tensor_mul(vv[:], dd2[:], brow(avh[:]))
            # poly+merge: M = (gamma*RA + beta)*RA + V   (alpha added later)
            h1 = singles.tile([P, 5, PP], F16, tag="h1")
            nc.vector.tensor_mul(h1[:], brow(gam[:]), ramp[:])
            nc.vector.tensor_add(h1[:], h1[:], brow(bet[:]))
            nc.vector.tensor_mul(h1[:], h1[:], ramp[:])
            mm = singles.tile([P, 5, PP], F16, tag="mm")
            nc.vector.tensor_add(mm[:], h1[:], vv[:])

            # min over a (tree), + alpha (f32), clamp at 0
            t01 = singles.tile([P, 2, PP], F16, tag="t01")
            nc.vector.tensor_tensor(out=t01[:], in0=mm[:, 0:2, :],
                                    in1=mm[:, 2:4, :],
                                    op=mybir.AluOpType.min)
            t2 = singles.tile([P, PP], F16, tag="t2")
            nc.vector.tensor_tensor(out=t2[:], in0=t01[:, 0, :],
                                    in1=t01[:, 1, :],
                                    op=mybir.AluOpType.min)
            m5 = singles.tile([P, PP], F16, tag="m5")
            nc.vector.tensor_tensor(out=m5[:], in0=t2[:], in1=mm[:, 4, :],
                                    op=mybir.AluOpType.min)
            pda = singles.tile([P, PP], F32, tag="pda")
            nc.vector.tensor_add(pda[:], m5[:], alph[:])
            pdist = singles.tile([P, PP], F32, tag="pdist")
            nc.vector.tensor_scalar(
                out=pdist[:], in0=pda[:], scalar1=0.0, scalar2=0.0,
                op0=mybir.AluOpType.max, op1=mybir.AluOpType.add)

            # ---- fused stage C, split into pair-halves so sqrt (ACT),
            # rr/pen (DVE) and the PE matmul pipeline ----
            CH = 64
            dist = small.tile([P, PP], F32, tag="dist")
            rr = small.tile([P, PP], F32, tag="rr")
            pen = small.tile([P, PP], F16, tag="pen")
            for lo, hi in ((0, CH), (CH, PP)):
                nc.scalar.activation(out=dist[:, lo:hi],
                                     in_=pdist[:, lo:hi],
                                     func=mybir.ActivationFunctionType.Sqrt)
                nc.vector.scalar_tensor_tensor(
                    out=rr[:, lo:hi], in0=dist[:, lo:hi], scalar=-1.0,
                    in1=xtb[:, XO_PRC + lo : XO_PRC + hi],
                    op0=mybir.AluOpType.mult, op1=mybir.AluOpType.mult)
                nc.vector.tensor_scalar(
                    out=pen[:, lo:hi], in0=rr[:, lo:hi],
                    scalar1=1.0, scalar2=0.0,
                    op0=mybir.AluOpType.add, op1=mybir.AluOpType.max)

            # ---- stage D ----
            w2f = singles.tile([P, NL], F16, tag="w2f")
            nc.vector.tensor_copy(out=w2f[:],
                                  in_=xta[:, XO_WMTA : XO_WMTA + NL])
            mindc = small.tile([PP, B], F16, tag="mindc")
            nc.gpsimd.tensor_copy(out=mindc[:], in_=mindt[:])
            ps1 = psum.tile([PP, 512], F32)
            for lo, hi in ((0, CH), (CH, PP)):
                nc.tensor.matmul(ps1[lo:hi, 0:NL], pen[:, lo:hi], w2f[:],
                                 start=True, stop=True)
            pairsum = small.tile([PP, NL], F16, tag="pairsum")
            nc.vector.tensor_copy(out=pairsum[:], in_=ps1[:, 0:NL])
            ps2 = psum.tile([NL, 512], F32)
            nc.tensor.matmul(ps2[:, 0:B], pairsum[:], mindc[:],
                             start=True, stop=True)
            lout = small.tile([NL, B], F32, tag="lout")
            nc.vector.tensor_copy(out=lout[:], in_=ps2[:, 0:B])
            nc.sync.dma_start(out=out[:], in_=lout[:])

    return nc


def _prepare(inputs):
    x = np.ascontiguousarray(inputs["x"], dtype=np.float32)
    extent = np.asarray(inputs["extent"], dtype=np.float32)
    wfa = np.asarray(inputs["world_from_agent"], dtype=np.float32)
    speed = np.asarray(inputs["curr_speed"], dtype=np.float32)
    scene = np.asarray(inputs["scene_index"])

    R = wfa[:, :2, :2]
    tr = wfa[:, :2, 2]
    yaw_off = np.arctan2(R[:, 1, 0], R[:, 0, 0]).astype(np.float32)
    agt_rad = extent[:, 1] / 2.0
    cmax = extent[:, 0] / 2.0 - agt_rad
    su = (cmax / 2.0).astype(np.float32)          # disk spacing
    pd = (agt_rad[:, None] + agt_rad[None, :] + BUFFER_DIST).astype(np.float32)
    moving = (np.abs(speed) > SPEED_TH)

    _, starts, counts = np.unique(scene, return_index=True, return_counts=True)
    scenes = [(int(o), int(s)) for o, s in zip(starts, counts)]
    assert sum(s for _, s in scenes) == B
    for o, s in scenes:
        assert (scene[o : o + s] == scene[o]).all()

    pairs_i, pairs_j, pairs_w = [], [], []
    for (o, s, K) in _rects(scenes):
        for i in range(s):
            for k in range(1, K + 1):
                pairs_i.append(o + i)
                pairs_j.append(o + (i + k) % s)
                pairs_w.append(0.5 if (s % 2 == 0 and k == s // 2) else 1.0)
    pairs_i = np.array(pairs_i)
    pairs_j = np.array(pairs_j)
    pairs_w = np.array(pairs_w, dtype=np.float32)
    PP = len(pairs_i)

    sui = su[pairs_i]
    svj = su[pairs_j]
    const_rows = np.concatenate([
        2.0 * sui,                      # TWOSU
        sui * sui,                      # SU2
        1.0 / svj,                      # INVSV
        sui / svj,                      # SUDSV
        svj * svj,                      # AV
        1.0 / pd[pairs_i, pairs_j],     # PRC
    ]).astype(np.float32)

    mind_arr = np.zeros((PP, B), dtype=np.float16)
    mv = moving.astype(np.float32)
    for q in range(PP):
        mind_arr[q, pairs_i[q]] = mv[pairs_i[q]] * pairs_w[q]
        mind_arr[q, pairs_j[q]] = mv[pairs_j[q]] * pairs_w[q]

    twopi = 2.0 * np.pi
    geo = np.concatenate([
        R[:, 0, 0], R[:, 1, 0],          # gA
        R[:, 0, 1], R[:, 1, 1],          # gB
        tr[:, 0], tr[:, 1],              # gT
        2.0 + yaw_off / twopi, 2.25 + yaw_off / twopi,  # shifts2
    ]).astype(np.float32)

    w = DECAY_RATE ** np.arange(T, dtype=np.float32)
    w = w / w.sum()
    wmt = np.zeros((P, NL), dtype=np.float32)
    for nl in range(NL):
        wmt[nl * T : (nl + 1) * T, nl] = w / B

    XWB = _xin_width_b(PP)
    xinb_row = np.empty((P, XWB), dtype=np.float16)
    xinb_row[:, :] = const_rows[None, :].astype(np.float16)
    in_maps = []
    for c in range(NCORES):
        xs = x[:, c * NL : (c + 1) * NL, :, :]          # (B, NL, T, 6)
        xs = xs[..., [0, 1, 3]]                          # (B, NL, T, 3)
        xdat = xs.transpose(1, 2, 3, 0).reshape(P, 3 * B)
        xina = np.empty((P, XWA), dtype=np.float32)
        xina[:, 0 : 3 * B] = xdat
        xina[:, XO_GEO : XO_WMTA] = geo[None, :]
        xina[:, XO_WMTA:] = wmt
        in_maps.append({"xina": xina, "xinb": xinb_row, "mind": mind_arr})

    return scenes, PP, in_maps, moving


_CACHE = {}


def _get_nc(scenes, PP):
    key = (tuple(scenes), PP)
    if key not in _CACHE:
        _CACHE[key] = _build_nc(scenes, PP)
    return _CACHE[key]


def _run(inputs, trace=False):
    scenes, PP, in_maps, moving = _prepare(inputs)
    nc = _get_nc(scenes, PP)
    res = run_bass_kernel_spmd(nc, in_maps, core_ids=list(range(NCORES)),
                               trace=trace)
    outf = np.zeros((B, N), dtype=np.float32)
    for c in range(NCORES):
        lc = res.results[c]["loss"]                      # (NL, B)
        for nl in range(NL):
            outf[:, c * NL + nl] = lc[nl]
    return outf, res


def kernel(**inputs):
    outf, _ = _run(inputs, trace=False)
    return outf


def _ensure_ntff_hook():
    """Register the axon NTFF profile hook if the container's antenv lacks it."""
    try:
        from antenv.axon_hooks import get_axon_ntff_profile_hook  # noqa: F401
        return
    except ImportError:
        pass
    import types

    if "/root/.axon_site" not in sys.path:
        sys.path.insert(0, "/root/.axon_site")
    from trn_agent_boot.trn_boot import _ntff_profile_via_ctypes

    hook = _ntff_profile_via_ctypes("/opt/axon/libaxon_pjrt.so")
    mod = types.ModuleType("antenv.axon_hooks")
    mod.get_axon_ntff_profile_hook = lambda: hook
    mod.set_axon_ntff_profile_hook = lambda h: None
    sys.modules["antenv.axon_hooks"] = mod


def run_traced(inputs):
    """Correctness output + profiled exec time (ns) via NTFF trace."""
    _ensure_ntff_hook()
    outf, res = _run(inputs, trace=True)
    return outf, res.exec_time_ns
